# revision 1
# baseline (speedup 1.0000x reference)
"""Trainium2 Bass kernel for nn_FCNNShapeCounterValuationFunction.

Computes out[i] = 0.999 * a[i, int(z[i, 5])] for z:[B,32] f32, a:[B,16] f32.

Strategy (pure data parallel, 8 NeuronCores):
  - Shard rows across 8 cores (BC = B/8 rows each).
  - Per core, view rows as [128 partitions, BC/128] with per-partition
    contiguous blocks so every DMA descriptor is a large contiguous chunk
    (full HBM bandwidth; this problem is memory-bound: ~96 MB/core).
  - Per round of up to 512 rows/partition: z loads ride the SP HWDGE ring
    and a loads the ACT ring (two descriptor generators feeding the 16
    SDMA engines in parallel); ACT extracts the index column; DVE does the
    16-way gather as 16 fused scalar_tensor_tensor ops
    prod[:,k,:] = (idx == k) * a[:,:,k], an in-place contiguous
    binary-tree add over k, and the 0.999 scale; GpSimd (SWDGE) issues the
    output stores so no load engine ever waits on compute. All compute
    hides under the DMA stream (~96 MB/core at ~375-380 GB/s measured).
"""

import numpy as np

B = 4194304
D = 32
K = 16
ATTR = 5
SCALE = 0.999
N_CORES = 8
P = 128
BC = B // N_CORES  # 524288 rows per core
F = 256  # rows per partition per tile

_cache = {}


def _round_sizes(npp):
    """Compute rounds (rows/partition each). Mostly 512-row rounds to
    amortize DVE per-op overhead; the last 512 is split 256/128/128 to
    shorten the post-DMA compute tail."""
    assert npp % 512 == 0 and npp >= 512
    if npp == 512:
        return [256, 128, 128]
    return [512] * (npp // 512 - 1) + [256, 128, 128]


def _build(bc=BC, f=F):
    """Build + compile the per-core Bass program for bc rows."""
    from contextlib import ExitStack

    import concourse.tile as tile
    from concourse import bacc, mybir

    npp = bc // P  # rows per partition
    assert bc % P == 0
    rounds = _round_sizes(npp)

    nc = bacc.Bacc("TRN2", target_bir_lowering=False, debug=False, num_devices=N_CORES)
    z = nc.dram_tensor("z", [bc, D], mybir.dt.float32, kind="ExternalInput")
    a = nc.dram_tensor("a", [bc, K], mybir.dt.float32, kind="ExternalInput")
    out = nc.dram_tensor("out", [bc], mybir.dt.float32, kind="ExternalOutput")

    # Partition-major views: partition p owns rows [p*npp, (p+1)*npp) so each
    # partition's DMA chunk is contiguous in DRAM.
    zv = z.ap().rearrange("(p n) d -> p n d", p=P)
    av = a.ap().rearrange("(p n) k -> p n k", p=P)
    ov = out.ap().rearrange("(p n) -> p n", p=P)

    f32 = mybir.dt.float32
    eq = mybir.AluOpType.is_equal
    mult = mybir.AluOpType.mult
    add = mybir.AluOpType.add
    FZ = 256  # rows/partition per z DMA tile (32 KB/partition chunks)

    with ExitStack() as ctx:
        tc = ctx.enter_context(tile.TileContext(nc))
        zpool = ctx.enter_context(tc.tile_pool(name="zpool", bufs=2))
        apool = ctx.enter_context(tc.tile_pool(name="apool", bufs=2))
        ppool = ctx.enter_context(tc.tile_pool(name="ppool", bufs=1))
        spool = ctx.enter_context(tc.tile_pool(name="spool", bufs=2))

        pos = 0
        for f in rounds:
            lo, hi = pos, pos + f
            pos = hi

            # a rides the ACT HWDGE ring, z the SP ring: two rings generate
            # descriptors in parallel so the 16 SDMA engines interleave both
            # streams at packet granularity.
            at = apool.tile([P, f, K], f32, tag="at", name="at")
            nc.scalar.dma_start(at[:], av[:, lo:hi, :])

            # z arrives in <=FZ-row tiles; idx collects the index column.
            idx = spool.tile([P, f], f32, tag="idx", name="idx")
            for zlo in range(lo, hi, FZ):
                zhi = min(zlo + FZ, hi)
                zt = zpool.tile([P, zhi - zlo, D], f32, tag="zt", name="zt")
                nc.sync.dma_start(zt[:], zv[:, zlo:zhi, :])
                nc.scalar.copy(idx[:, zlo - lo : zhi - lo], zt[:, :, ATTR])

            # prod[:, k, :] = (idx == k) * a[:, :, k]   (k-major: contiguous out)
            prod = ppool.tile([P, K, f], f32, tag="prod", name="prod")
            for k in range(K):
                nc.vector.scalar_tensor_tensor(
                    prod[:, k, :], idx[:], float(k), at[:, :, k], eq, mult
                )

            # In-place binary-tree sum over k: all operands contiguous.
            for h in (8, 4, 2):
                nc.vector.tensor_tensor(
                    prod[:, :h, :], prod[:, :h, :], prod[:, h : 2 * h, :], add
                )
            red = spool.tile([P, f], f32, tag="red", name="red")
            nc.vector.tensor_tensor(red[:], prod[:, 0, :], prod[:, 1, :], add)

            # Scale on DVE (single-src fp32 runs in 2x mode) and store via
            # GpSimd SWDGE: ACT's in-order queue then carries only loads and
            # idx extracts, so no next-round work ever waits on this round's
            # compute, and the compute tail after the last load is minimal.
            sc = spool.tile([P, f], f32, tag="sc", name="sc")
            nc.vector.tensor_scalar_mul(sc[:], red[:], SCALE)
            nc.gpsimd.dma_start(ov[:, lo:hi], sc[:])

    nc.compile()
    return nc


def _get(bc=BC, f=F):
    key = (bc, f)
    if key not in _cache:
        _cache[key] = _build(bc, f)
    return _cache[key]


def kernel(z, a, attr_index=5, **run_kwargs):
    """Full inputs in, full output out. Shards rows over 8 NeuronCores."""
    from concourse import bass_utils

    assert int(attr_index) == ATTR
    z = np.asarray(z, dtype=np.float32)
    a = np.asarray(a, dtype=np.float32)
    assert z.shape == (B, D) and a.shape == (B, K)

    nc = _get()
    in_maps = [
        {"z": z[c * BC : (c + 1) * BC], "a": a[c * BC : (c + 1) * BC]}
        for c in range(N_CORES)
    ]
    res = bass_utils.run_bass_kernel_spmd(
        nc, in_maps, core_ids=list(range(N_CORES)), **run_kwargs
    )
    out = np.concatenate([r["out"] for r in res.results], axis=0)
    if run_kwargs:
        kernel.last_results = res
    return out

